# revision 2
# baseline (speedup 1.0000x reference)
"""Trainium2 Bass kernel for segment-reduce classifier (v2).

Reference computation:
    local = relu(x @ Wloc.T)            # [L, 128]
    feats = local.reshape(-1, 30, 128).mean(1)   # [L/30, 128]
    out   = feats @ W.T                 # [L/30, 10]

The kernel is PSUM-drain bound: every local element (fp32 in PSUM) must be
relu'd + copied to SBUF by ScalarE (1.2 GHz) or VectorE (0.96 GHz), each
limited to 1 elem/lane/cycle from PSUM (GPSIMD and DMA have no PSUM port).
Combined floor ~2.16 G elem/s/lane -> ~70us/core for 150000 elems/lane.

v2 design (per core, data-parallel rows):
  - x shard host-cast to fp8e4 and host-permuted so PSUM output is already
    j-major per 510-col chunk (17 segments x 30 offsets); the whole shard
    [128, 75480] stays resident in SBUF (cols padded with zeros), loaded by
    8 chunked DMAs so compute starts after ~1us.
  - mm1: per chunk, two concurrent K=64 row-group matmuls (fp16 Wloc x fp8 x)
    fill one 2-bank PSUM tile [128, 1024] fp32 (A chunk bank0, B chunk bank1).
  - drain: ONE relu instruction per tile, FD=1020, contiguous reads+writes,
    greedily assigned to ScalarE/VectorE by modeled cost to balance busy time.
  - mm2 (pool+classifier): per group of 15 chunks, 30 accumulating j-matmuls
    (rhs [128,2,15,17] slices of rl), C=10 packed 4x into PE column strips;
    acc [128, <=510] fp32 in 1 PSUM bank. Host sums the 4 strips.
  - acc drained per group into out staging; compact [40, cols] strip DMAs out
    per group so there is no output tail.
"""

import numpy as np
import ml_dtypes

import concourse.bacc as bacc
import concourse.bass as bass
import concourse.tile as tile
from concourse import mybir
from concourse.bass_utils import run_bass_kernel_spmd

# Problem constants (hardcoded per harness contract)
L, D_IN, D_ENC, C, J = 1200000, 64, 128, 10, 30
N_CORES = 8
R = L // N_CORES            # rows per core = 150000
HALF = R // 2               # 75000 rows per half-stream
SEG_H = HALF // J           # 2500 real segments per half
CH = 510                    # chunk cols = 17 segments * 30
GSEG = CH // J              # 17 segments per chunk per half
NCHUNK = 148                # ceil(75000/510) -> padded to 148*510
COLS = NCHUNK * CH          # 75480 padded cols per half
GROUPS = [15] * 9 + [13]    # chunks per mm2 accumulation group (sum=148)
SLOTS_H = NCHUNK * GSEG     # 2516 segment slots per half (incl. 16 bogus)
OUT_COLS = 2 * GSEG * sum(GROUPS)  # 5032 staged output cols
# j-subsets for the 4 PE column-group strips of the classifier matmul
J_SETS = [list(range(0, 8)), list(range(8, 16)),
          list(range(16, 23)), list(range(23, 30))]

_CACHE = {}

# modeled per-drain-instruction cost (ns) for greedy engine balancing
def _act_cost(fd):
    return (300.0 + fd) / 1.2

def _dve_cost(fd):
    return (120.0 + fd) / 0.96


def _build_kernel():
    nc = bacc.Bacc("TRN2", target_bir_lowering=False, debug=False,
                   num_devices=N_CORES)
    f32, f16, f8 = mybir.dt.float32, mybir.dt.float16, mybir.dt.float8e4

    xt_d = nc.dram_tensor("xt", [128, COLS], f8, kind="ExternalInput")
    w1_d = nc.dram_tensor("w1", [128, D_ENC], f16, kind="ExternalInput")
    w2_d = nc.dram_tensor("w2", [128, C], f16, kind="ExternalInput")
    out_d = nc.dram_tensor("out", [40, OUT_COLS], f32, kind="ExternalOutput")

    with tile.TileContext(nc) as tc:
        with (
            tc.tile_pool(name="consts", bufs=1) as consts,
            tc.tile_pool(name="xres", bufs=1) as xres,
            tc.tile_pool(name="rlp", bufs=2) as rlp,
            tc.tile_pool(name="outp", bufs=1) as outp,
            tc.tile_pool(name="psp", bufs=3, space="PSUM") as psp,
            tc.tile_pool(name="accp", bufs=2, space="PSUM") as accp,
        ):
            w1 = consts.tile([128, D_ENC], f16)
            nc.sync.dma_start(w1[:], w1_d[:])
            w2 = consts.tile([128, C], f16)
            nc.sync.dma_start(w2[:], w2_d[:])

            xt = xres.tile([128, COLS], f8)
            # chunk-aligned input segments; small first one for fast start
            seg_bounds = [0, 4, 20, 36, 52, 76, 100, 124, 148]
            for a, b in zip(seg_bounds, seg_bounds[1:]):
                nc.sync.dma_start(xt[:, a * CH:b * CH], xt_d[:, a * CH:b * CH])

            out_sb = outp.tile([128, OUT_COLS], f32)

            t_act = 0.0
            t_dve = 0.0

            def drain(rout, pin, fd):
                nonlocal t_act, t_dve
                ca, cd = _act_cost(fd), _dve_cost(fd)
                if t_act + ca <= t_dve + cd:
                    t_act += ca
                    nc.scalar.activation(rout, pin,
                                         mybir.ActivationFunctionType.Relu)
                else:
                    t_dve += cd
                    nc.vector.tensor_scalar_max(rout, pin, 0.0)

            # group state
            gidx = 0
            gstart = 0            # first chunk of current group
            rl = None
            rlv = None
            pending = None        # (rl, nch, ocol) awaiting mm2 emission
            ocol = 0

            def emit_mm2(p_rl, nch, p_ocol):
                n = 2 * nch * GSEG
                rhs_all = p_rl.rearrange("p (h q) -> p h q", h=2)[
                    :, :, 0:nch * CH].rearrange(
                    "p h (c j g) -> p h c j g", c=nch, j=J)
                acc = accp.tile([128, 512], f32, tag="acc")
                for k in range(8):
                    for s in range(4):
                        if k >= len(J_SETS[s]):
                            continue
                        j = J_SETS[s][k]
                        nc.tensor.matmul(acc[32 * s:32 * s + C, 0:n],
                                         w2[:], rhs_all[:, :, :, j, :],
                                         start=(k == 0),
                                         stop=(k == len(J_SETS[s]) - 1),
                                         tile_position=(0, 32 * s))
                # drain accumulator -> staging, then compact strip DMAs out
                drain_dst = out_sb[:, p_ocol:p_ocol + n]
                nonlocal t_act, t_dve
                ca, cd = _act_cost(n), _dve_cost(n)
                if t_act + ca <= t_dve + cd:
                    t_act += ca
                    nc.scalar.copy(drain_dst, acc[:, 0:n])
                else:
                    t_dve += cd
                    nc.vector.tensor_copy(drain_dst, acc[:, 0:n])
                for s in range(4):
                    nc.sync.dma_start(
                        out_d[10 * s:10 * s + 10, p_ocol:p_ocol + n],
                        out_sb[32 * s:32 * s + 10, p_ocol:p_ocol + n])

            for c in range(NCHUNK):
                if c == gstart:
                    rl = rlp.tile([128, 2 * 15 * CH], f16, tag="rl")
                    rlv = rl.rearrange("p (h q) -> p h q", h=2)
                c0 = c * CH
                pp = psp.tile([128, 1024], f32, tag="pp")
                nc.tensor.matmul(pp[:, 0:CH], w1[0:64, :], xt[0:64, c0:c0 + CH])
                nc.tensor.matmul(pp[:, 512:512 + CH], w1[64:128, :],
                                 xt[64:128, c0:c0 + CH])
                lc = c - gstart
                pin = pp.rearrange("p (h q) -> p h q", h=2)[:, :, 0:CH]
                rout = rlv[:, :, lc * CH:(lc + 1) * CH]
                drain(rout, pin, 2 * CH)

                # defer mm2 of the previous group until the drains it needs
                # have certainly retired (psp rotation lags ~3 chunks)
                if pending is not None and lc >= 4:
                    emit_mm2(*pending)
                    pending = None
                if lc == GROUPS[gidx] - 1:
                    pending = (rl, GROUPS[gidx], ocol)
                    ocol += 2 * GROUPS[gidx] * GSEG
                    gstart = c + 1
                    gidx += 1
            emit_mm2(*pending)

    nc.compile()
    return nc


def _pack_inputs(x, Wloc, W):
    x = np.asarray(x, dtype=np.float32)
    # [core, half, row, d] -> pad rows to 148*510 -> j-major per 510-chunk
    xp = x.reshape(N_CORES, 2, HALF, D_IN)
    pad = np.zeros((N_CORES, 2, COLS - HALF, D_IN), dtype=np.float32)
    xp = np.concatenate([xp, pad], axis=2)
    # within each chunk: source row = g*30 + j  ->  column j*17 + g
    xp = xp.reshape(N_CORES, 2, NCHUNK, GSEG, J, D_IN)
    xp = xp.transpose(0, 1, 2, 4, 3, 5)          # [., c, j, g, d]
    xp = xp.reshape(N_CORES, 2, COLS, D_IN)
    xp = xp.transpose(0, 1, 3, 2)                # [core, half, d, col]
    xp = np.ascontiguousarray(xp).reshape(N_CORES, 128, COLS)
    xp = xp.astype(ml_dtypes.float8_e4m3fn)

    w1 = np.ascontiguousarray(
        np.concatenate([Wloc.T, Wloc.T], axis=0), dtype=np.float16)
    w2 = np.ascontiguousarray((W / float(J)).T, dtype=np.float16)
    return xp, w1, w2


def kernel(x: np.ndarray, Wloc: np.ndarray, W: np.ndarray) -> np.ndarray:
    if "nc" not in _CACHE:
        _CACHE["nc"] = _build_kernel()
    nc = _CACHE["nc"]

    xp, w1, w2 = _pack_inputs(x, Wloc, W)
    in_maps = [{"xt": xp[c], "w1": w1, "w2": w2} for c in range(N_CORES)]
    res = run_bass_kernel_spmd(nc, in_maps, core_ids=list(range(N_CORES)))
    _CACHE["exec_time_ns"] = res.exec_time_ns
    _CACHE["trace"] = res.instructions_and_trace

    out = np.empty((L // J, C), dtype=np.float32)
    for core in range(N_CORES):
        od = res.results[core]["out"]            # [40, OUT_COLS]
        oc = od[0:10] + od[10:20] + od[20:30] + od[30:40]   # [10, 5032]
        seg_vals = np.empty((2, SLOTS_H, C), dtype=np.float32)
        off = 0
        cstart = 0
        for nch in GROUPS:
            n = 2 * nch * GSEG
            blk = oc[:, off:off + n].reshape(C, 2, nch * GSEG)
            s0 = cstart * GSEG
            seg_vals[0, s0:s0 + nch * GSEG] = blk[:, 0].T
            seg_vals[1, s0:s0 + nch * GSEG] = blk[:, 1].T
            off += n
            cstart += nch
        base = core * (R // J)
        out[base:base + SEG_H] = seg_vals[0, :SEG_H]
        out[base + SEG_H:base + 2 * SEG_H] = seg_vals[1, :SEG_H]
    return out


# revision 7
# speedup vs baseline: 1.0570x; 1.0570x over previous
"""Trainium2 Bass kernel for segment-reduce classifier (v2).

Reference computation:
    local = relu(x @ Wloc.T)            # [L, 128]
    feats = local.reshape(-1, 30, 128).mean(1)   # [L/30, 128]
    out   = feats @ W.T                 # [L/30, 10]

The kernel is PSUM-drain bound: every local element (fp32 in PSUM) must be
relu'd + copied to SBUF by ScalarE (1.2 GHz) or VectorE (0.96 GHz), each
limited to 1 elem/lane/cycle from PSUM (GPSIMD and DMA have no PSUM port).
Combined floor ~2.16 G elem/s/lane -> ~70us/core for 150000 elems/lane.

v2 design (per core, data-parallel rows):
  - x shard host-cast to fp8e4 and host-permuted so PSUM output is already
    j-major per 510-col chunk (17 segments x 30 offsets); the whole shard
    [128, 75480] stays resident in SBUF (cols padded with zeros), loaded by
    8 chunked DMAs so compute starts after ~1us.
  - mm1: per chunk, two concurrent K=64 row-group matmuls (fp16 Wloc x fp8 x)
    fill one 2-bank PSUM tile [128, 1024] fp32 (A chunk bank0, B chunk bank1).
  - drain: ONE relu instruction per tile, FD=1020, contiguous reads+writes,
    greedily assigned to ScalarE/VectorE by modeled cost to balance busy time.
  - mm2 (pool+classifier): per group of 15 chunks, 30 accumulating j-matmuls
    (rhs [128,2,15,17] slices of rl), C=10 packed 4x into PE column strips;
    acc [128, <=510] fp32 in 1 PSUM bank. Host sums the 4 strips.
  - acc drained per group into out staging; compact [40, cols] strip DMAs out
    per group so there is no output tail.
"""

import numpy as np
import ml_dtypes

import concourse.bacc as bacc
import concourse.bass as bass
import concourse.tile as tile
from concourse import mybir
from concourse.bass_utils import run_bass_kernel_spmd

# Problem constants (hardcoded per harness contract)
L, D_IN, D_ENC, C, J = 1200000, 64, 128, 10, 30
N_CORES = 8
R = L // N_CORES            # rows per core = 150000
HALF = R // 2               # 75000 rows per half-stream
SEG_H = HALF // J           # 2500 real segments per half
CH = 510                    # chunk cols = 17 segments * 30
GSEG = CH // J              # 17 segments per chunk per half
NCHUNK = 148                # ceil(75000/510) -> padded to 148*510
COLS = NCHUNK * CH          # 75480 padded cols per half
GROUPS = [15] * 9 + [13]    # chunks per mm2 accumulation group (sum=148)
SLOTS_H = NCHUNK * GSEG     # 2516 segment slots per half (incl. 16 bogus)
OUT_COLS = 2 * GSEG * sum(GROUPS)  # 5032 staged output cols
# j-subsets for the 4 PE column-group strips of the classifier matmul
J_SETS = [list(range(0, 8)), list(range(8, 16)),
          list(range(16, 23)), list(range(23, 30))]

_CACHE = {}

# modeled per-drain-instruction cost (ns) for greedy engine balancing
def _act_cost(fd):
    return (300.0 + fd) / 1.2

def _dve_cost(fd):
    return (120.0 + fd) / 0.96


def _build_kernel():
    nc = bacc.Bacc("TRN2", target_bir_lowering=False, debug=False,
                   num_devices=N_CORES)
    f32, f16, f8 = mybir.dt.float32, mybir.dt.float16, mybir.dt.float8e4

    xt_d = nc.dram_tensor("xt", [128, COLS], f8, kind="ExternalInput")
    w1_d = nc.dram_tensor("w1", [128, D_ENC], f8, kind="ExternalInput")
    w2_d = nc.dram_tensor("w2", [128, C], f16, kind="ExternalInput")
    out_d = nc.dram_tensor("out", [40, OUT_COLS], f32, kind="ExternalOutput")

    with tile.TileContext(nc) as tc:
        with (
            tc.tile_pool(name="consts", bufs=1) as consts,
            tc.tile_pool(name="xres", bufs=1) as xres,
            tc.tile_pool(name="rlp", bufs=2) as rlp,
            tc.tile_pool(name="outp", bufs=1) as outp,
            tc.tile_pool(name="psp", bufs=3, space="PSUM") as psp,
            tc.tile_pool(name="accp", bufs=2, space="PSUM") as accp,
        ):
            w1 = consts.tile([128, D_ENC], f8)
            nc.sync.dma_start(w1[:], w1_d[:])
            w2 = consts.tile([128, C], f16)
            nc.sync.dma_start(w2[:], w2_d[:])

            xt = xres.tile([128, COLS], f8)
            # chunk-aligned input segments; small first one for fast start
            seg_bounds = [0, 4, 20, 36, 52, 76, 100, 124, 148]
            for a, b in zip(seg_bounds, seg_bounds[1:]):
                nc.sync.dma_start(xt[:, a * CH:b * CH], xt_d[:, a * CH:b * CH])

            out_sb = outp.tile([128, OUT_COLS], f32)

            t_act = 0.0
            t_dve = 0.0

            def drain(rout, pin, fd):
                nonlocal t_act, t_dve
                ca, cd = _act_cost(fd), _dve_cost(fd)
                if t_act + ca <= t_dve + cd:
                    t_act += ca
                    nc.scalar.activation(rout, pin,
                                         mybir.ActivationFunctionType.Relu)
                else:
                    t_dve += cd
                    nc.vector.tensor_scalar_max(rout, pin, 0.0)

            # group state
            gidx = 0
            gstart = 0            # first chunk of current group
            rl = None
            rlv = None
            pending = None        # [rl, nch, ocol, acc] awaiting mm2 k-slices
            ocol = 0

            def mm2_kslice(pend, k):
                p_rl, nch, _, acc = pend
                n = 2 * nch * GSEG
                rhs_all = p_rl.rearrange("p (h q) -> p h q", h=2)[
                    :, :, 0:nch * CH].rearrange(
                    "p h (c j g) -> p h c j g", c=nch, j=J)
                for s in range(4):
                    if k >= len(J_SETS[s]):
                        continue
                    j = J_SETS[s][k]
                    nc.tensor.matmul(acc[32 * s:32 * s + C, 0:n],
                                     w2[:], rhs_all[:, :, :, j, :],
                                     start=(k == 0),
                                     stop=(k == len(J_SETS[s]) - 1),
                                     tile_position=(0, 32 * s))

            def mm2_finish(pend):
                # drain accumulator -> staging, then compact strip DMAs out
                _, nch, p_ocol, acc = pend
                n = 2 * nch * GSEG
                nonlocal t_act, t_dve
                drain_dst = out_sb[:, p_ocol:p_ocol + n]
                ca, cd = _act_cost(n), _dve_cost(n)
                if t_act + ca <= t_dve + cd:
                    t_act += ca
                    nc.scalar.copy(drain_dst, acc[:, 0:n])
                else:
                    t_dve += cd
                    nc.vector.tensor_copy(drain_dst, acc[:, 0:n])
                for s in range(4):
                    nc.sync.dma_start(
                        out_d[10 * s:10 * s + 10, p_ocol:p_ocol + n],
                        out_sb[32 * s:32 * s + 10, p_ocol:p_ocol + n])

            for c in range(NCHUNK):
                if c == gstart:
                    rl = rlp.tile([128, 2 * 15 * CH], f16, tag="rl")
                    rlv = rl.rearrange("p (h q) -> p h q", h=2)
                c0 = c * CH
                pp = psp.tile([128, 1024], f32, tag="pp")
                nc.tensor.matmul(pp[:, 0:CH], w1[0:64, :], xt[0:64, c0:c0 + CH])
                nc.tensor.matmul(pp[:, 512:512 + CH], w1[64:128, :],
                                 xt[64:128, c0:c0 + CH])
                lc = c - gstart
                pin = pp.rearrange("p (h q) -> p h q", h=2)[:, :, 0:CH]
                rout = rlv[:, :, lc * CH:(lc + 1) * CH]
                drain(rout, pin, 2 * CH)

                # previous group's classifier matmuls: one k-slice (4 strip
                # matmuls, ~0.4us PE) per chunk so the PE never starves the
                # relu engines with a long mm2 burst
                if pending is not None and 1 <= lc <= 9:
                    if lc == 1:
                        pending[3] = accp.tile([128, 512], f32, tag="acc", name="acc")
                    if lc <= 8:
                        mm2_kslice(pending, lc - 1)
                    else:
                        mm2_finish(pending)
                        pending = None
                if lc == GROUPS[gidx] - 1:
                    pending = [rl, GROUPS[gidx], ocol, None]
                    ocol += 2 * GROUPS[gidx] * GSEG
                    gstart = c + 1
                    gidx += 1
            pending[3] = accp.tile([128, 512], f32, tag="acc", name="acc")
            for k in range(8):
                mm2_kslice(pending, k)
            mm2_finish(pending)

    nc.compile()
    return nc


def _pack_inputs(x, Wloc, W):
    x = np.asarray(x, dtype=np.float32)
    # [core, half, row, d] -> pad rows to 148*510 -> j-major per 510-chunk
    xp = x.reshape(N_CORES, 2, HALF, D_IN)
    pad = np.zeros((N_CORES, 2, COLS - HALF, D_IN), dtype=np.float32)
    xp = np.concatenate([xp, pad], axis=2)
    # within each chunk: source row = g*30 + j  ->  column j*17 + g
    xp = xp.reshape(N_CORES, 2, NCHUNK, GSEG, J, D_IN)
    xp = xp.transpose(0, 1, 2, 4, 3, 5)          # [., c, j, g, d]
    xp = xp.reshape(N_CORES, 2, COLS, D_IN)
    xp = xp.transpose(0, 1, 3, 2)                # [core, half, d, col]
    xp = np.ascontiguousarray(xp).reshape(N_CORES, 128, COLS)
    xp = xp.astype(ml_dtypes.float8_e4m3fn)

    w1 = np.concatenate([Wloc.T, Wloc.T], axis=0).astype(
        ml_dtypes.float8_e4m3fn)
    w2 = np.ascontiguousarray((W / float(J)).T, dtype=np.float16)
    return xp, w1, w2


def kernel(x: np.ndarray, Wloc: np.ndarray, W: np.ndarray) -> np.ndarray:
    if "nc" not in _CACHE:
        _CACHE["nc"] = _build_kernel()
    nc = _CACHE["nc"]

    xp, w1, w2 = _pack_inputs(x, Wloc, W)
    in_maps = [{"xt": xp[c], "w1": w1, "w2": w2} for c in range(N_CORES)]
    res = run_bass_kernel_spmd(nc, in_maps, core_ids=list(range(N_CORES)))
    _CACHE["exec_time_ns"] = res.exec_time_ns
    _CACHE["trace"] = res.instructions_and_trace

    out = np.empty((L // J, C), dtype=np.float32)
    for core in range(N_CORES):
        od = res.results[core]["out"]            # [40, OUT_COLS]
        oc = od[0:10] + od[10:20] + od[20:30] + od[30:40]   # [10, 5032]
        seg_vals = np.empty((2, SLOTS_H, C), dtype=np.float32)
        off = 0
        cstart = 0
        for nch in GROUPS:
            n = 2 * nch * GSEG
            blk = oc[:, off:off + n].reshape(C, 2, nch * GSEG)
            s0 = cstart * GSEG
            seg_vals[0, s0:s0 + nch * GSEG] = blk[:, 0].T
            seg_vals[1, s0:s0 + nch * GSEG] = blk[:, 1].T
            off += n
            cstart += nch
        base = core * (R // J)
        out[base:base + SEG_H] = seg_vals[0, :SEG_H]
        out[base + SEG_H:base + 2 * SEG_H] = seg_vals[1, :SEG_H]
    return out


# revision 10
# speedup vs baseline: 1.1713x; 1.1081x over previous
"""Trainium2 Bass kernel for segment-reduce classifier (v2).

Reference computation:
    local = relu(x @ Wloc.T)            # [L, 128]
    feats = local.reshape(-1, 30, 128).mean(1)   # [L/30, 128]
    out   = feats @ W.T                 # [L/30, 10]

The kernel is PSUM-drain bound: every local element (fp32 in PSUM) must be
relu'd + copied to SBUF by ScalarE (1.2 GHz) or VectorE (0.96 GHz), each
limited to 1 elem/lane/cycle from PSUM (GPSIMD and DMA have no PSUM port).
Combined floor ~2.16 G elem/s/lane -> ~70us/core for 150000 elems/lane.

v2 design (per core, data-parallel rows):
  - x shard host-cast to fp8e4 and host-permuted so PSUM output is already
    j-major per 510-col chunk (17 segments x 30 offsets); the whole shard
    [128, 75480] stays resident in SBUF (cols padded with zeros), loaded by
    8 chunked DMAs so compute starts after ~1us.
  - mm1: per chunk, two concurrent K=64 row-group matmuls (fp16 Wloc x fp8 x)
    fill one 2-bank PSUM tile [128, 1024] fp32 (A chunk bank0, B chunk bank1).
  - drain: ONE relu instruction per tile, FD=1020, contiguous reads+writes,
    greedily assigned to ScalarE/VectorE by modeled cost to balance busy time.
  - mm2 (pool+classifier): per group of 15 chunks, 30 accumulating j-matmuls
    (rhs [128,2,15,17] slices of rl), C=10 packed 4x into PE column strips;
    acc [128, <=510] fp32 in 1 PSUM bank. Host sums the 4 strips.
  - acc drained per group into out staging; compact [40, cols] strip DMAs out
    per group so there is no output tail.
"""

import numpy as np
import ml_dtypes

import concourse.bacc as bacc
import concourse.bass as bass
import concourse.tile as tile
from concourse import mybir
from concourse.bass_utils import run_bass_kernel_spmd

# Problem constants (hardcoded per harness contract)
L, D_IN, D_ENC, C, J = 1200000, 64, 128, 10, 30
N_CORES = 8
R = L // N_CORES            # rows per core = 150000
HALF = R // 2               # 75000 rows per half-stream
SEG_H = HALF // J           # 2500 real segments per half
CH = 510                    # chunk cols = 17 segments * 30
GSEG = CH // J              # 17 segments per chunk per half
NCHUNK = 148                # ceil(75000/510) -> padded to 148*510
COLS = NCHUNK * CH          # 75480 padded cols per half
GROUPS = [15] * 9 + [13]    # chunks per mm2 accumulation group (sum=148)
SLOTS_H = NCHUNK * GSEG     # 2516 segment slots per half (incl. 16 bogus)
OUT_COLS = 2 * GSEG * sum(GROUPS)  # 5032 staged output cols
# j-subsets for the 4 PE column-group strips of the classifier matmul
J_SETS = [list(range(0, 8)), list(range(8, 16)),
          list(range(16, 23)), list(range(23, 30))]

_CACHE = {}

# modeled per-drain-instruction cost (ns) for greedy engine balancing
def _act_cost(fd):
    return (300.0 + fd) / 1.2

def _dve_cost(fd):
    return (120.0 + fd) / 0.96


def _build_kernel():
    nc = bacc.Bacc("TRN2", target_bir_lowering=False, debug=False,
                   num_devices=N_CORES)
    f32, f16, f8 = mybir.dt.float32, mybir.dt.float16, mybir.dt.float8e4

    xt_d = nc.dram_tensor("xt", [128, COLS], f8, kind="ExternalInput")
    w1_d = nc.dram_tensor("w1", [128, D_ENC], f8, kind="ExternalInput")
    w2_d = nc.dram_tensor("w2", [128, C], f16, kind="ExternalInput")
    out_d = nc.dram_tensor("out", [40, OUT_COLS], f32, kind="ExternalOutput")

    with tile.TileContext(nc) as tc:
        with (
            tc.tile_pool(name="consts", bufs=1) as consts,
            tc.tile_pool(name="xres", bufs=1) as xres,
            tc.tile_pool(name="rlp", bufs=2) as rlp,
            tc.tile_pool(name="outp", bufs=1) as outp,
            tc.tile_pool(name="psp", bufs=3, space="PSUM") as psp,
            tc.tile_pool(name="accp", bufs=2, space="PSUM") as accp,
        ):
            w1 = consts.tile([128, D_ENC], f8)
            nc.sync.dma_start(w1[:], w1_d[:])
            w2 = consts.tile([128, C], f16)
            nc.sync.dma_start(w2[:], w2_d[:])

            xt = xres.tile([128, COLS], f8)
            # chunk-aligned input segments; small first ones for fast start
            seg_bounds = [0, 2, 6, 14, 26, 46, 80, 114, 148]
            for a, b in zip(seg_bounds, seg_bounds[1:]):
                nc.sync.dma_start(xt[:, a * CH:b * CH], xt_d[:, a * CH:b * CH])

            out_sb = outp.tile([128, OUT_COLS], f32)

            t_act = 0.0
            t_dve = 0.0

            def drain(rout, pin, fd):
                nonlocal t_act, t_dve
                ca, cd = _act_cost(fd), _dve_cost(fd)
                if t_act + ca <= t_dve + cd:
                    t_act += ca
                    nc.scalar.activation(rout, pin,
                                         mybir.ActivationFunctionType.Relu)
                else:
                    t_dve += cd
                    nc.vector.tensor_scalar_max(rout, pin, 0.0)

            # group state
            gidx = 0
            gstart = 0            # first chunk of current group
            rl = None
            rlv = None
            pending = None        # [rl, nch, ocol, acc] awaiting mm2 k-slices
            ocol = 0

            def mm2_kslice(pend, k):
                p_rl, nch, _, acc = pend
                n = 2 * nch * GSEG
                rhs_all = p_rl.rearrange("p (h q) -> p h q", h=2)[
                    :, :, 0:nch * CH].rearrange(
                    "p h (c j g) -> p h c j g", c=nch, j=J)
                for s in range(4):
                    if k >= len(J_SETS[s]):
                        continue
                    j = J_SETS[s][k]
                    nc.tensor.matmul(acc[32 * s:32 * s + C, 0:n],
                                     w2[:], rhs_all[:, :, :, j, :],
                                     start=(k == 0),
                                     stop=(k == len(J_SETS[s]) - 1),
                                     tile_position=(0, 32 * s))

            def mm2_finish(pend):
                # drain accumulator -> staging, then compact strip DMAs out
                _, nch, p_ocol, acc = pend
                n = 2 * nch * GSEG
                nonlocal t_act, t_dve
                drain_dst = out_sb[:, p_ocol:p_ocol + n]
                ca, cd = _act_cost(n), _dve_cost(n)
                if t_act + ca <= t_dve + cd:
                    t_act += ca
                    nc.scalar.copy(drain_dst, acc[:, 0:n])
                else:
                    t_dve += cd
                    nc.vector.tensor_copy(drain_dst, acc[:, 0:n])
                for s in range(4):
                    nc.sync.dma_start(
                        out_d[10 * s:10 * s + 10, p_ocol:p_ocol + n],
                        out_sb[32 * s:32 * s + 10, p_ocol:p_ocol + n])

            # k-slice schedule within the next group: every other chunk so
            # the cold PE keeps slack over the drain cadence (denser map for
            # the short 13-chunk final group)
            KSLOT15 = {1: 0, 3: 1, 5: 2, 7: 3, 9: 4, 11: 5, 13: 6, 14: 7}
            KSLOT13 = {1: 0, 2: 1, 3: 2, 4: 3, 5: 4, 7: 5, 9: 6, 11: 7}

            for c in range(NCHUNK):
                if c == gstart:
                    rl = rlp.tile([128, 2 * 15 * CH], f16, tag="rl")
                    rlv = rl.rearrange("p (h q) -> p h q", h=2)
                lc = c - gstart
                KSLOT = KSLOT15 if GROUPS[gidx] == 15 else KSLOT13
                # previous group's classifier matmuls: one k-slice (4 strip
                # matmuls) emitted BEFORE this chunk's encoder pair so the
                # scheduler cannot wedge it between the pair's two matmuls
                fin = False
                if pending is not None and lc in KSLOT:
                    if KSLOT[lc] == 0:
                        pending[3] = accp.tile([128, 512], f32, tag="acc",
                                               name="acc")
                    mm2_kslice(pending, KSLOT[lc])
                    if KSLOT[lc] == 7:
                        fin = True
                c0 = c * CH
                pp = psp.tile([128, 1024], f32, tag="pp")
                nc.tensor.matmul(pp[:, 0:CH], w1[0:64, :], xt[0:64, c0:c0 + CH])
                nc.tensor.matmul(pp[:, 512:512 + CH], w1[64:128, :],
                                 xt[64:128, c0:c0 + CH])
                pin = pp.rearrange("p (h q) -> p h q", h=2)[:, :, 0:CH]
                rout = rlv[:, :, lc * CH:(lc + 1) * CH]
                drain(rout, pin, 2 * CH)
                if fin:
                    mm2_finish(pending)
                    pending = None
                if lc == GROUPS[gidx] - 1:
                    pending = [rl, GROUPS[gidx], ocol, None]
                    ocol += 2 * GROUPS[gidx] * GSEG
                    gstart = c + 1
                    gidx += 1
            pending[3] = accp.tile([128, 512], f32, tag="acc", name="acc")
            for k in range(8):
                mm2_kslice(pending, k)
            mm2_finish(pending)

    nc.compile()
    return nc


def _pack_inputs(x, Wloc, W):
    x = np.asarray(x, dtype=np.float32)
    # [core, half, row, d] -> pad rows to 148*510 -> j-major per 510-chunk
    xp = x.reshape(N_CORES, 2, HALF, D_IN)
    pad = np.zeros((N_CORES, 2, COLS - HALF, D_IN), dtype=np.float32)
    xp = np.concatenate([xp, pad], axis=2)
    # within each chunk: source row = g*30 + j  ->  column j*17 + g
    xp = xp.reshape(N_CORES, 2, NCHUNK, GSEG, J, D_IN)
    xp = xp.transpose(0, 1, 2, 4, 3, 5)          # [., c, j, g, d]
    xp = xp.reshape(N_CORES, 2, COLS, D_IN)
    xp = xp.transpose(0, 1, 3, 2)                # [core, half, d, col]
    xp = np.ascontiguousarray(xp).reshape(N_CORES, 128, COLS)
    xp = xp.astype(ml_dtypes.float8_e4m3fn)

    w1 = np.concatenate([Wloc.T, Wloc.T], axis=0).astype(
        ml_dtypes.float8_e4m3fn)
    w2 = np.ascontiguousarray((W / float(J)).T, dtype=np.float16)
    return xp, w1, w2


def kernel(x: np.ndarray, Wloc: np.ndarray, W: np.ndarray) -> np.ndarray:
    if "nc" not in _CACHE:
        _CACHE["nc"] = _build_kernel()
    nc = _CACHE["nc"]

    xp, w1, w2 = _pack_inputs(x, Wloc, W)
    in_maps = [{"xt": xp[c], "w1": w1, "w2": w2} for c in range(N_CORES)]
    res = run_bass_kernel_spmd(nc, in_maps, core_ids=list(range(N_CORES)))
    _CACHE["exec_time_ns"] = res.exec_time_ns
    _CACHE["trace"] = res.instructions_and_trace

    out = np.empty((L // J, C), dtype=np.float32)
    for core in range(N_CORES):
        od = res.results[core]["out"]            # [40, OUT_COLS]
        oc = od[0:10] + od[10:20] + od[20:30] + od[30:40]   # [10, 5032]
        seg_vals = np.empty((2, SLOTS_H, C), dtype=np.float32)
        off = 0
        cstart = 0
        for nch in GROUPS:
            n = 2 * nch * GSEG
            blk = oc[:, off:off + n].reshape(C, 2, nch * GSEG)
            s0 = cstart * GSEG
            seg_vals[0, s0:s0 + nch * GSEG] = blk[:, 0].T
            seg_vals[1, s0:s0 + nch * GSEG] = blk[:, 1].T
            off += n
            cstart += nch
        base = core * (R // J)
        out[base:base + SEG_H] = seg_vals[0, :SEG_H]
        out[base + SEG_H:base + 2 * SEG_H] = seg_vals[1, :SEG_H]
    return out
